# revision 3
# baseline (speedup 1.0000x reference)
"""Trainium2 Bass kernel for nn_AtomicConvScore (MoE-routing style).

Strategy (routed / expert-grouped, data-parallel over atoms):
  * Concatenate frag1/frag2/complex atoms into one list with a per-atom
    sign (+1 complex, -1 frags); the answer is
        out[b] = sum_n sign[n] * MLP_{z[n]}(x[n])   (+ bout correction)
  * Sort atoms by type on the host, pad each type group to a multiple of
    8*128 and give each core 1/8 of every type group -> every core runs
    the IDENTICAL instruction schedule (SPMD) on different data.
  * Per 512-atom supertile, four pipeline stages on the PE:
      A: h1T = W0^T x       (4 matmuls, bf16, fp32 PSUM)
      B: h2T = W1^T h1      (4 matmuls)
      C: p3T[atom,k] = h2^T W2 per 128-atom tile (stationary = h2 chunk)
         -> L3 output lands atom-major with zero transpose cost
      D: G[k,b] += h3T_j^T @ S_j  (signed batch-mask reduction in PSUM,
         one tiny matmul per tile; per-type G evacuated to SBUF)
  * Software pipelining: the PE instruction stream is emitted as
    A[i], B[i-1], C[i-2], D[i-3] so PSUM->SBUF relu evacuations
    (split across the Act and DVE engines) never stall the PE and its
    DVFS clock stays ramped at 2.4 GHz.
  * PSUM plan (8 banks): P1 2x[128,1024] (4), P2 2x[128,1024] (2... see
    pools below), P3 1x[128,512], G 1x[128,16].
  * DMA issue is spread across the SP and Act HWDGE queues so the first
    weights + x chunk land ~3.5us in; PE warmup matmuls (independent of
    the activation-table preload) keep the clock ramping meanwhile.
  * Final fold (sum_k Wout[t,k] * G_t[k,b], sum over types/cores, bout)
    happens on the host from the per-type G matrices.
"""

import os
import sys

sys.path.insert(0, "/opt/trn_rl_repo")

import numpy as np
import ml_dtypes

import concourse.bass as bass
import concourse.tile as tile
from concourse import bacc, mybir
from concourse.bass_utils import run_bass_kernel_spmd

# Problem constants (hardcoded per the self-contained-kernel contract).
B = 16
F = 256
H1, H2, H3 = 256, 256, 128
T = 5
NCORES = 8
PTILE = 128  # atoms per tile (partition dim)
SUPER = 4    # tiles per supertile -> moving dim N = 512
N_WARMUP = int(os.environ.get("KWARM", "5"))
KDBG = set(filter(None, os.environ.get("KDBG", "").split(",")))

MM_MODE = "bf16-pipe"  # informational (printed by test.py)
TRACE = False          # test.py sets this for profiling runs
LAST_RESULTS = None    # test.py reads exec_time_ns from here

_F32 = mybir.dt.float32
_BF16 = mybir.dt.bfloat16

# CONSTW column layout per type t:
#   +0    w0 chunks (k,m) at (k*2+m)*128
#   +512  w1 chunks (k,m)
#   +1024 w2 chunks (k)
_TBLK = 1280
_WCOLS = T * _TBLK
# CONSTF: 25 bias cols (only DMA'd when b0/b1 nonzero)
_FCOLS = 25


def _schedule(k_t):
    """supertile schedule [(t, nt, jg)] with jg = global tile index."""
    sched = []
    jg = 0
    for t in range(T):
        left = int(k_t[t])
        while left > 0:
            nt = min(SUPER, left)
            sched.append((t, nt, jg))
            jg += nt
            left -= nt
    return sched


def _build(k_t, n_core, has_b01, has_b2):
    """Build the (SPMD-uniform) Bass program for one core."""
    ntt = n_core // PTILE
    relu = mybir.ActivationFunctionType.Relu
    sched = _schedule(k_t)
    n_st = len(sched)

    active = sorted({t for t, _, _ in sched})
    first_st = {t: min(i for i, (tt, _, _) in enumerate(sched) if tt == t)
                for t in active}
    last_st = {t: max(i for i, (tt, _, _) in enumerate(sched) if tt == t)
               for t in active}

    # x chunk plan: boundaries in supertile indices
    bounds = [0]
    for step in (2, 4):
        if bounds[-1] + step < n_st:
            bounds.append(bounds[-1] + step)
    bounds.append(n_st)
    # per-supertile (chunk id, atom col offset inside chunk)
    st_chunk = {}
    chunk_atoms = []
    for ci in range(len(bounds) - 1):
        off = 0
        for s in range(bounds[ci], bounds[ci + 1]):
            st_chunk[s] = (ci, off)
            off += sched[s][1] * PTILE
        chunk_atoms.append(off)
    n_chunks = len(chunk_atoms)

    nc = bacc.Bacc()
    xT_d = nc.dram_tensor("xT", [F, n_core], _BF16, kind="ExternalInput")
    CW_d = nc.dram_tensor("CONSTW", [PTILE, _WCOLS], _BF16, kind="ExternalInput")
    S2_d = nc.dram_tensor("S2", [PTILE, ntt * B], _BF16, kind="ExternalInput")
    CF_d = None
    if has_b01:
        CF_d = nc.dram_tensor("CONSTF", [PTILE, _FCOLS], _F32,
                              kind="ExternalInput")
    B2_d = None
    if has_b2:
        B2_d = nc.dram_tensor("B2R", [1, PTILE + T * H3], _BF16,
                              kind="ExternalInput")
    out_d = nc.dram_tensor("res", [PTILE, T * B], _F32, kind="ExternalOutput")

    with tile.TileContext(nc) as tc:
        with (
            tc.tile_pool(name="const", bufs=1) as cpool,
            tc.tile_pool(name="x", bufs=1) as xpool,
            tc.tile_pool(name="h", bufs=2) as hpool,
            tc.tile_pool(name="p1", bufs=2, space="PSUM") as p1pool,
            tc.tile_pool(name="p2", bufs=1, space="PSUM") as p2pool,
            tc.tile_pool(name="p3", bufs=1, space="PSUM") as p3pool,
            tc.tile_pool(name="pg", bufs=1, space="PSUM") as pgpool,
        ):
            # ---- warm tiles (DVE memsets; independent of DMAs) ----
            warm1 = cpool.tile([128, 128], _F32, tag="warm1")
            nc.vector.memset(warm1[:], 0.0)
            warm2 = cpool.tile([128, 2], _F32, tag="warm2")
            nc.vector.memset(warm2[:], 0.0)

            # ---- PE warmup: start the DVFS ramp while DMAs stream in.
            # fp32 (4 cyc/row) so each one is long; values never read.
            wps = p1pool.tile([128, 1024], _F32, tag="p1", name="wps")
            for _ in range(N_WARMUP):
                nc.tensor.matmul(wps[:, 0:128], warm1[:], warm1[:],
                                 start=True, stop=True)

            # ---- DMA issue, spread across the two HWDGE queues ----
            # scalar (Act) queue: x chunks, then the act-table preload
            x0c = []
            x1c = []
            for ci in range(n_chunks):
                cA = chunk_atoms[ci]
                x0 = xpool.tile([128, cA], _BF16, tag=f"x0c{ci}",
                                name=f"x0c{ci}")
                x1 = xpool.tile([128, cA], _BF16, tag=f"x1c{ci}",
                                name=f"x1c{ci}")
                x0c.append(x0)
                x1c.append(x1)
            col = 0
            for ci in range(n_chunks):
                cA = chunk_atoms[ci]
                nc.scalar.dma_start(x0c[ci][:], xT_d[0:128, col:col + cA])
                nc.scalar.dma_start(x1c[ci][:], xT_d[128:256, col:col + cA])
                col += cA
                if ci == 1 or n_chunks == 1:
                    # activation-table preload once the head-critical x
                    # chunks are issued
                    nc.scalar.activation(warm2[:, 0:1], warm2[:, 1:2], relu)

            # sync (SP) queue: weights (t0 w0 first), S2, biases
            t0 = active[0]
            CWt = {t: cpool.tile([PTILE, _TBLK], _BF16, tag=f"CW{t}",
                                 name=f"CW{t}")
                   for t in active}
            nc.sync.dma_start(CWt[t0][:, 0:512],
                              CW_d[:, t0 * _TBLK:t0 * _TBLK + 512])
            nc.sync.dma_start(CWt[t0][:, 512:_TBLK],
                              CW_d[:, t0 * _TBLK + 512:(t0 + 1) * _TBLK])
            S2 = cpool.tile([PTILE, ntt * B], _BF16, tag="S2")
            nc.sync.dma_start(S2[:], S2_d[:])
            for t in active[1:]:
                nc.sync.dma_start(CWt[t][:],
                                  CW_d[:, t * _TBLK:(t + 1) * _TBLK])
            CF = None
            if has_b01:
                CF = cpool.tile([PTILE, _FCOLS], _F32, tag="CF")
                nc.sync.dma_start(CF[:], CF_d[:])
            B2R = None
            if has_b2:
                B2R = cpool.tile([1, PTILE + T * H3], _BF16, tag="B2R")
                nc.sync.dma_start(B2R[:], B2_d[:])

            def w0(t, k, m):
                c = (k * 2 + m) * 128
                return CWt[t][:, c:c + 128]

            def w1(t, k, m):
                c = 512 + (k * 2 + m) * 128
                return CWt[t][:, c:c + 128]

            def w2(t, k):
                c = 1024 + k * 128
                return CWt[t][:, c:c + 128]

            def bias(t, c):
                return CF[:, t * 5 + c:t * 5 + c + 1]

            GSB = cpool.tile([PTILE, T * B], _F32, tag="GSB")

            # per-supertile state carried between pipeline stages
            p1t = [None] * n_st
            h1t = [None] * n_st
            p2t = [None] * n_st
            h2t = [None] * n_st
            p3t = [None] * n_st
            h3t = [None] * n_st
            G = [None]  # current type's PSUM accumulator

            def ev_relu(eng, dst, src, b):
                """PSUM->SBUF relu (+bias) on the chosen engine."""
                if eng == 0:
                    if b is None:
                        nc.scalar.activation(dst, src, relu)
                    else:
                        nc.scalar.activation(dst, src, relu, bias=b)
                else:
                    if b is None:
                        nc.vector.tensor_scalar_max(dst, src, 0.0)
                    else:
                        nc.vector.tensor_scalar(
                            dst, src, b, 0.0,
                            mybir.AluOpType.add, mybir.AluOpType.max)

            def stage_A(i):
                t, nt, _ = sched[i]
                N = nt * PTILE
                ci, off = st_chunk[i]
                xs0 = x0c[ci][:, off:off + N]
                xs1 = x1c[ci][:, off:off + N]
                p1 = p1pool.tile([128, 1024], _F32, tag="p1", name="p1")
                p1t[i] = p1
                for m in range(2):
                    o = m * 512
                    nc.tensor.matmul(p1[:, o:o + N], w0(t, 0, m), xs0,
                                     start=True, stop=False)
                    nc.tensor.matmul(p1[:, o:o + N], w0(t, 1, m), xs1,
                                     start=False, stop=True)

            def stage_E1(i):
                t, nt, _ = sched[i]
                N = nt * PTILE
                p1 = p1t[i]
                h1 = hpool.tile([128, 1024], _BF16, tag="h1", name="h1")
                h1t[i] = h1
                b0a = bias(t, 0) if has_b01 else None
                b0b = bias(t, 1) if has_b01 else None
                ev_relu(0, h1[:, 0:N], p1[:, 0:N], b0a)
                ev_relu(1, h1[:, 512:512 + N], p1[:, 512:512 + N], b0b)
                p1t[i] = None

            def stage_B(i):
                t, nt, _ = sched[i]
                N = nt * PTILE
                h1 = h1t[i]
                p2 = p2pool.tile([128, 1024], _F32, tag="p2", name="p2")
                p2t[i] = p2
                for m in range(2):
                    o = m * 512
                    nc.tensor.matmul(p2[:, o:o + N], w1(t, 0, m),
                                     h1[:, 0:N], start=True, stop=False)
                    nc.tensor.matmul(p2[:, o:o + N], w1(t, 1, m),
                                     h1[:, 512:512 + N], start=False,
                                     stop=True)

            def stage_E2(i):
                t, nt, _ = sched[i]
                N = nt * PTILE
                p2 = p2t[i]
                h2 = hpool.tile([128, 1024], _BF16, tag="h2", name="h2")
                h2t[i] = h2
                b1a = bias(t, 2) if has_b01 else None
                b1b = bias(t, 3) if has_b01 else None
                ev_relu(1, h2[:, 0:N], p2[:, 0:N], b1a)
                ev_relu(0, h2[:, 512:512 + N], p2[:, 512:512 + N], b1b)
                p2t[i] = None
                h1t[i] = None

            def stage_C(i):
                t, nt, _ = sched[i]
                h2 = h2t[i]
                p3 = p3pool.tile([128, 512], _F32, tag="p3", name="p3")
                p3t[i] = p3
                for j in range(nt):
                    jc = j * PTILE
                    reg = p3[:, jc:jc + PTILE]
                    nc.tensor.matmul(reg, h2[:, jc:jc + PTILE], w2(t, 0),
                                     start=True, stop=False)
                    nc.tensor.matmul(reg, h2[:, 512 + jc:512 + jc + PTILE],
                                     w2(t, 1), start=False, stop=not has_b2)
                    if has_b2:
                        c0 = PTILE + t * H3
                        nc.tensor.matmul(reg, B2R[0:1, 0:PTILE],
                                         B2R[0:1, c0:c0 + H3],
                                         start=False, stop=True)

            def stage_E3(i):
                t, nt, _ = sched[i]
                N = nt * PTILE
                p3 = p3t[i]
                h3 = hpool.tile([128, 512], _BF16, tag="h3t", name="h3")
                h3t[i] = h3
                ev_relu(i % 2, h3[:, 0:N], p3[:, 0:N], None)
                p3t[i] = None
                h2t[i] = None

            def stage_D(i):
                t, nt, jg = sched[i]
                h3 = h3t[i]
                if i == first_st[t]:
                    G[0] = pgpool.tile([H3, B], _F32, tag="G", name="G")
                if "nogp" not in KDBG:
                    for j in range(nt):
                        jj = jg + j
                        nc.tensor.matmul(
                            G[0][:], h3[:, j * PTILE:(j + 1) * PTILE],
                            S2[:, jj * B:(jj + 1) * B],
                            start=(i == first_st[t] and j == 0),
                            stop=(i == last_st[t] and j == nt - 1),
                            skip_group_check=True)
                if i == last_st[t]:
                    nc.scalar.copy(GSB[:, t * B:(t + 1) * B], G[0][:])
                    G[0] = None
                h3t[i] = None

            # ---- software-pipelined main loop ----
            for i in range(n_st + 3):
                if i < n_st:
                    stage_A(i)
                    stage_E1(i)
                if 1 <= i <= n_st:
                    stage_B(i - 1)
                    stage_E2(i - 1)
                if 2 <= i <= n_st + 1:
                    stage_C(i - 2)
                    stage_E3(i - 2)
                if 3 <= i <= n_st + 2:
                    stage_D(i - 3)

            # ---- final: ship the per-type G matrices ----
            if "nogp" in KDBG:
                nc.vector.memset(GSB[:], 0.0)
            nc.sync.dma_start(out_d[:], GSB[:])
    nc.finalize()
    return nc


def kernel(**inputs):
    global LAST_RESULTS
    f1 = np.ascontiguousarray(np.asarray(inputs["frag1_layer"], np.float32))
    f2 = np.ascontiguousarray(np.asarray(inputs["frag2_layer"], np.float32))
    cx = np.ascontiguousarray(np.asarray(inputs["complex_layer"], np.float32))
    z1 = np.asarray(inputs["frag1_z"])
    z2 = np.asarray(inputs["frag2_z"])
    zc = np.asarray(inputs["complex_z"])
    W0 = np.asarray(inputs["W0"], np.float32)
    b0 = np.asarray(inputs["b0"], np.float32)
    W1 = np.asarray(inputs["W1"], np.float32)
    b1 = np.asarray(inputs["b1"], np.float32)
    W2 = np.asarray(inputs["W2"], np.float32)
    b2 = np.asarray(inputs["b2"], np.float32)
    Wout = np.asarray(inputs["Wout"], np.float32)
    bout = np.asarray(inputs["bout"], np.float32)

    np_dt = ml_dtypes.bfloat16
    has_b01 = bool(np.any(b0 != 0.0) or np.any(b1 != 0.0))
    has_b2 = bool(np.any(b2 != 0.0))

    x_all = np.concatenate([f1, f2, cx], axis=1)          # [B, Na, F]
    z_all = np.concatenate([z1, z2, zc], axis=1)          # [B, Na]
    Bn, Na, _ = x_all.shape
    assert Bn == B
    sgn = np.concatenate([
        np.full(f1.shape[1], -1.0, np.float32),
        np.full(f2.shape[1], -1.0, np.float32),
        np.full(cx.shape[1], 1.0, np.float32),
    ])

    xf = x_all.reshape(-1, F)
    zf = z_all.reshape(-1).astype(np.int64)
    bidx = np.repeat(np.arange(B), Na)
    sf = np.tile(sgn, B)

    order = np.argsort(zf, kind="stable")
    counts = np.bincount(zf, minlength=T)[:T]
    GRAN = NCORES * PTILE
    padded = -(-counts // GRAN) * GRAN
    k_t = (padded // GRAN).astype(int)
    n_core = int(padded.sum()) // NCORES
    ntt = n_core // PTILE

    # Per-core atom index lists; -1 marks padding.
    per_core = [[] for _ in range(NCORES)]
    pos = 0
    for t in range(T):
        ct, pt = int(counts[t]), int(padded[t])
        idx = order[pos:pos + ct]
        pos += ct
        if pt == 0:
            continue
        ip = np.full(pt, -1, np.int64)
        ip[:ct] = idx
        ip = ip.reshape(NCORES, pt // NCORES)
        for c in range(NCORES):
            per_core[c].append(ip[c])
    idx_cores = np.stack([np.concatenate(l) for l in per_core])  # [NC, n]

    valid = idx_cores >= 0
    safe = np.where(valid, idx_cores, 0)
    xg = xf[safe]
    xg[~valid] = 0.0
    xT = np.ascontiguousarray(xg.transpose(0, 2, 1)).astype(np_dt)  # [NC,F,n]

    # S[c, n, b] = sign * (batch == b)
    S = np.zeros((NCORES, n_core, B), np.float32)
    rows = sf[safe] * valid
    bcols = bidx[safe]
    S[np.arange(NCORES)[:, None], np.arange(n_core)[None, :], bcols] = rows

    # CONSTW: weights packed per type in the _TBLK layout
    CWh = np.zeros((PTILE, _WCOLS), np.float32)
    for t in range(T):
        base = t * _TBLK
        for k in range(2):
            for m in range(2):
                CWh[:, base + (k * 2 + m) * 128:base + (k * 2 + m + 1) * 128] = \
                    W0[t, 128 * k:128 * (k + 1), 128 * m:128 * (m + 1)]
                CWh[:, base + 512 + (k * 2 + m) * 128:
                    base + 512 + (k * 2 + m + 1) * 128] = \
                    W1[t, 128 * k:128 * (k + 1), 128 * m:128 * (m + 1)]
            CWh[:, base + 1024 + k * 128:base + 1024 + (k + 1) * 128] = \
                W2[t, 128 * k:128 * (k + 1), 0:128]
    CWh = np.ascontiguousarray(CWh).astype(np_dt)

    # CONSTF: 25 bias cols (b0 halves, b1 halves, b2 first half)
    CFh = np.zeros((PTILE, _FCOLS), np.float32)
    for t in range(T):
        CFh[:, t * 5 + 0] = b0[t, :128]
        CFh[:, t * 5 + 1] = b0[t, 128:]
        CFh[:, t * 5 + 2] = b1[t, :128]
        CFh[:, t * 5 + 3] = b1[t, 128:]
        CFh[:, t * 5 + 4] = b2[t, :128]

    bias_term = np.bincount(bidx, weights=(sf * bout[zf, 0]).astype(np.float64),
                            minlength=B)[:B]

    nc = _build(k_t, n_core, has_b01, has_b2)
    in_maps = []
    for c in range(NCORES):
        # S2[p, j*B + b] (tile-major)
        s2 = np.ascontiguousarray(
            S[c].reshape(ntt, PTILE, B).transpose(1, 0, 2)
        ).reshape(PTILE, ntt * B).astype(np_dt)
        m = {"xT": xT[c], "CONSTW": CWh, "S2": s2}
        if has_b01:
            m["CONSTF"] = CFh
        if has_b2:
            b2r = np.zeros((1, PTILE + T * H3), np.float32)
            b2r[0, :PTILE] = 1.0
            b2r[0, PTILE:] = b2[:, :H3].reshape(-1)
            m["B2R"] = b2r.astype(np_dt)
        in_maps.append(m)

    kw = {}
    if TRACE:
        kw = dict(trace=True, trace_cores=list(range(NCORES)))
    res = run_bass_kernel_spmd(nc, in_maps, core_ids=list(range(NCORES)), **kw)
    LAST_RESULTS = res

    # host fold: out[b] = sum_c sum_t sum_k Wout[t,k] * G[c,t,k,b] + bias
    out = bias_term.copy()
    wo = Wout[:, :, 0].astype(np.float64)  # [T, H3]
    for c in range(NCORES):
        g = res.results[c]["res"].reshape(PTILE, T, B).astype(np.float64)
        out += np.einsum("tk,ktb->b", wo, g[:H3])
    return out.astype(np.float32)[:, None]


# revision 10
# speedup vs baseline: 1.0296x; 1.0296x over previous
"""Trainium2 Bass kernel for nn_AtomicConvScore (MoE-routing style).

Strategy (routed / expert-grouped, data-parallel over atoms):
  * Concatenate frag1/frag2/complex atoms into one list with a per-atom
    sign (+1 complex, -1 frags); the answer is
        out[b] = sum_n sign[n] * MLP_{z[n]}(x[n])   (+ bout correction)
  * Sort atoms by type on the host, pad each type group to a multiple of
    8*128 and give each core 1/8 of every type group -> every core runs
    the IDENTICAL instruction schedule (SPMD) on different data.
  * Per 512-atom supertile, four pipeline stages on the PE:
      A: h1T = W0^T x       (4 matmuls, bf16, fp32 PSUM)
      B: h2T = W1^T h1      (4 matmuls)
      C: p3T[atom,k] = h2^T W2 per 128-atom tile (stationary = h2 chunk)
         -> L3 output lands atom-major with zero transpose cost
      D: G[k,b] += h3T_j^T @ S_j  (signed batch-mask reduction in PSUM,
         one tiny matmul per tile; per-type G evacuated to SBUF)
  * Software pipelining: the PE instruction stream is emitted as
    A[i], B[i-1], C[i-2], D[i-3] so PSUM->SBUF relu evacuations
    (split across the Act and DVE engines) never stall the PE and its
    DVFS clock stays ramped at 2.4 GHz.
  * PSUM plan (8 banks): P1 2x[128,1024] (4), P2 2x[128,1024] (2... see
    pools below), P3 1x[128,512], G 1x[128,16].
  * DMA issue is spread across the SP and Act HWDGE queues so the first
    weights + x chunk land ~3.5us in; PE warmup matmuls (independent of
    the activation-table preload) keep the clock ramping meanwhile.
  * Final fold (sum_k Wout[t,k] * G_t[k,b], sum over types/cores, bout)
    happens on the host from the per-type G matrices.
"""

import os
import sys

sys.path.insert(0, "/opt/trn_rl_repo")

import numpy as np
import ml_dtypes

import concourse.bass as bass
import concourse.tile as tile
from concourse import bacc, mybir
from concourse.bass_utils import run_bass_kernel_spmd

# Problem constants (hardcoded per the self-contained-kernel contract).
B = 16
F = 256
H1, H2, H3 = 256, 256, 128
T = 5
NCORES = 8
PTILE = 128  # atoms per tile (partition dim)
SUPER = 4    # tiles per supertile -> moving dim N = 512
N_WARMUP = int(os.environ.get("KWARM", "6"))
KDBG = set(filter(None, os.environ.get("KDBG", "").split(",")))

MM_MODE = "bf16-pipe"  # informational (printed by test.py)
TRACE = False          # test.py sets this for profiling runs
LAST_RESULTS = None    # test.py reads exec_time_ns from here

_F32 = mybir.dt.float32
_BF16 = mybir.dt.bfloat16

# CONSTW column layout per type t:
#   +0    w0 chunks (k,m) at (k*2+m)*128
#   +512  w1 chunks (k,m)
#   +1024 w2 chunks (k)
_TBLK = 1280
_WCOLS = T * _TBLK
# CONSTF: 25 bias cols (only DMA'd when b0/b1 nonzero)
_FCOLS = 25


def _schedule(k_t):
    """supertile schedule [(t, nt, jg)] with jg = global tile index."""
    sched = []
    jg = 0
    for t in range(T):
        left = int(k_t[t])
        while left > 0:
            nt = min(SUPER, left)
            sched.append((t, nt, jg))
            jg += nt
            left -= nt
    return sched


def _build(k_t, n_core, has_b01, has_b2):
    """Build the (SPMD-uniform) Bass program for one core."""
    ntt = n_core // PTILE
    relu = mybir.ActivationFunctionType.Relu
    sched = _schedule(k_t)
    n_st = len(sched)

    active = sorted({t for t, _, _ in sched})
    first_st = {t: min(i for i, (tt, _, _) in enumerate(sched) if tt == t)
                for t in active}
    last_st = {t: max(i for i, (tt, _, _) in enumerate(sched) if tt == t)
               for t in active}

    # x chunk plan: boundaries in supertile indices
    bounds = [0]
    for step in (2, 4):
        if bounds[-1] + step < n_st:
            bounds.append(bounds[-1] + step)
    bounds.append(n_st)
    # per-supertile (chunk id, atom col offset inside chunk)
    st_chunk = {}
    chunk_atoms = []
    for ci in range(len(bounds) - 1):
        off = 0
        for s in range(bounds[ci], bounds[ci + 1]):
            st_chunk[s] = (ci, off)
            off += sched[s][1] * PTILE
        chunk_atoms.append(off)
    n_chunks = len(chunk_atoms)

    nc = bacc.Bacc()
    xT_d = nc.dram_tensor("xT", [F, n_core], _BF16, kind="ExternalInput")
    CW_d = nc.dram_tensor("CONSTW", [PTILE, _WCOLS], _BF16, kind="ExternalInput")
    S2_d = nc.dram_tensor("S2", [PTILE, ntt * B], _BF16, kind="ExternalInput")
    CF_d = None
    if has_b01:
        CF_d = nc.dram_tensor("CONSTF", [PTILE, _FCOLS], _F32,
                              kind="ExternalInput")
    B2_d = None
    if has_b2:
        B2_d = nc.dram_tensor("B2R", [1, PTILE + T * H3], _BF16,
                              kind="ExternalInput")
    out_d = nc.dram_tensor("res", [PTILE, T * B], _F32, kind="ExternalOutput")

    with tile.TileContext(nc) as tc:
        with (
            tc.tile_pool(name="const", bufs=1) as cpool,
            tc.tile_pool(name="x", bufs=1) as xpool,
            tc.tile_pool(name="h", bufs=3) as hpool,
            tc.tile_pool(name="p1", bufs=2, space="PSUM") as p1pool,
            tc.tile_pool(name="p2", bufs=1, space="PSUM") as p2pool,
            tc.tile_pool(name="p3", bufs=1, space="PSUM") as p3pool,
            tc.tile_pool(name="pg", bufs=1, space="PSUM") as pgpool,
        ):
            # ---- warm tile (DVE memset; independent of DMAs) ----
            warm1 = cpool.tile([128, 128], _F32, tag="warm1")
            nc.vector.memset(warm1[:], 0.0)

            # ---- PE warmup: start the DVFS ramp while DMAs stream in.
            # fp32 (4 cyc/row) so each one is long; values never read.
            wps = p1pool.tile([128, 1024], _F32, tag="p1", name="wps")
            for _ in range(N_WARMUP):
                nc.tensor.matmul(wps[:, 0:128], warm1[:], warm1[:],
                                 start=True, stop=True)

            # ---- DMA issue, spread across the two HWDGE queues ----
            x0c = []
            x1c = []
            for ci in range(n_chunks):
                cA = chunk_atoms[ci]
                x0 = xpool.tile([128, cA], _BF16, tag=f"x0c{ci}",
                                name=f"x0c{ci}")
                x1 = xpool.tile([128, cA], _BF16, tag=f"x1c{ci}",
                                name=f"x1c{ci}")
                x0c.append(x0)
                x1c.append(x1)
            # chunk-0 halves issue in parallel: x0c0 on scalar, x1c0 on sync
            c0A = chunk_atoms[0]
            nc.scalar.dma_start(x0c[0][:], xT_d[0:128, 0:c0A])
            nc.sync.dma_start(x1c[0][:], xT_d[128:256, 0:c0A])
            t0 = active[0]
            CWt = {t: cpool.tile([PTILE, _TBLK], _BF16, tag=f"CW{t}",
                                 name=f"CW{t}")
                   for t in active}
            nc.sync.dma_start(CWt[t0][:, 0:512],
                              CW_d[:, t0 * _TBLK:t0 * _TBLK + 512])
            col = c0A
            for ci in range(1, n_chunks):
                cA = chunk_atoms[ci]
                nc.scalar.dma_start(x0c[ci][:], xT_d[0:128, col:col + cA])
                nc.scalar.dma_start(x1c[ci][:], xT_d[128:256, col:col + cA])
                col += cA
            nc.sync.dma_start(CWt[t0][:, 512:_TBLK],
                              CW_d[:, t0 * _TBLK + 512:(t0 + 1) * _TBLK])
            S2 = cpool.tile([PTILE, ntt * B], _BF16, tag="S2")
            nc.sync.dma_start(S2[:], S2_d[:])
            for t in active[1:]:
                nc.sync.dma_start(CWt[t][:],
                                  CW_d[:, t * _TBLK:(t + 1) * _TBLK])
            CF = None
            if has_b01:
                CF = cpool.tile([PTILE, _FCOLS], _F32, tag="CF")
                nc.sync.dma_start(CF[:], CF_d[:])
            B2R = None
            if has_b2:
                B2R = cpool.tile([1, PTILE + T * H3], _BF16, tag="B2R")
                nc.sync.dma_start(B2R[:], B2_d[:])

            def w0(t, k, m):
                c = (k * 2 + m) * 128
                return CWt[t][:, c:c + 128]

            def w1(t, k, m):
                c = 512 + (k * 2 + m) * 128
                return CWt[t][:, c:c + 128]

            def w2(t, k):
                c = 1024 + k * 128
                return CWt[t][:, c:c + 128]

            def bias(t, c):
                return CF[:, t * 5 + c:t * 5 + c + 1]

            GSB = cpool.tile([PTILE, T * B], _F32, tag="GSB")

            # per-supertile state carried between pipeline stages
            p1t = [None] * n_st
            h1t = [None] * n_st
            p2t = [None] * n_st
            h2t = [None] * n_st
            p3t = [None] * n_st
            h3t = [None] * n_st
            G = [None]  # current type's PSUM accumulator

            def ev_relu(eng, dst, src, b):
                """PSUM->SBUF relu (+bias) on the chosen engine."""
                if eng == 0:
                    if b is None:
                        nc.scalar.activation(dst, src, relu)
                    else:
                        nc.scalar.activation(dst, src, relu, bias=b)
                else:
                    if b is None:
                        nc.vector.tensor_scalar_max(dst, src, 0.0)
                    else:
                        nc.vector.tensor_scalar(
                            dst, src, b, 0.0,
                            mybir.AluOpType.add, mybir.AluOpType.max)

            def stage_A(i):
                t, nt, _ = sched[i]
                N = nt * PTILE
                ci, off = st_chunk[i]
                xs0 = x0c[ci][:, off:off + N]
                xs1 = x1c[ci][:, off:off + N]
                p1 = p1pool.tile([128, 1024], _F32, tag="p1", name="p1")
                p1t[i] = p1
                for m in range(2):
                    o = m * 512
                    nc.tensor.matmul(p1[:, o:o + N], w0(t, 0, m), xs0,
                                     start=True, stop=False)
                    nc.tensor.matmul(p1[:, o:o + N], w0(t, 1, m), xs1,
                                     start=False, stop=True)

            def stage_E1(i, eng):
                t, nt, _ = sched[i]
                N = nt * PTILE
                p1 = p1t[i]
                h1 = hpool.tile([128, 1024], _BF16, tag="h1", name="h1")
                h1t[i] = h1
                if has_b01:
                    ev_relu(eng, h1[:, 0:N], p1[:, 0:N], bias(t, 0))
                    ev_relu(1 - eng, h1[:, 512:512 + N],
                            p1[:, 512:512 + N], bias(t, 1))
                else:
                    # one op over both halves (middle cols of partial
                    # supertiles carry unused garbage; B never reads them)
                    ev_relu(eng, h1[:, 0:512 + N], p1[:, 0:512 + N], None)
                p1t[i] = None

            def stage_B(i):
                t, nt, _ = sched[i]
                N = nt * PTILE
                h1 = h1t[i]
                p2 = p2pool.tile([128, 1024], _F32, tag="p2", name="p2")
                p2t[i] = p2
                for m in range(2):
                    o = m * 512
                    nc.tensor.matmul(p2[:, o:o + N], w1(t, 0, m),
                                     h1[:, 0:N], start=True, stop=False)
                    nc.tensor.matmul(p2[:, o:o + N], w1(t, 1, m),
                                     h1[:, 512:512 + N], start=False,
                                     stop=True)

            def stage_E2(i, eng):
                t, nt, _ = sched[i]
                N = nt * PTILE
                p2 = p2t[i]
                h2 = hpool.tile([128, 1024], _BF16, tag="h2", name="h2")
                h2t[i] = h2
                if has_b01:
                    ev_relu(eng, h2[:, 0:N], p2[:, 0:N], bias(t, 2))
                    ev_relu(1 - eng, h2[:, 512:512 + N],
                            p2[:, 512:512 + N], bias(t, 3))
                else:
                    ev_relu(eng, h2[:, 0:512 + N], p2[:, 0:512 + N], None)
                p2t[i] = None
                h1t[i] = None

            def stage_C(i):
                t, nt, _ = sched[i]
                h2 = h2t[i]
                p3 = p3pool.tile([128, 512], _F32, tag="p3", name="p3")
                p3t[i] = p3
                for j in range(nt):
                    jc = j * PTILE
                    reg = p3[:, jc:jc + PTILE]
                    nc.tensor.matmul(reg, h2[:, jc:jc + PTILE], w2(t, 0),
                                     start=True, stop=False)
                    nc.tensor.matmul(reg, h2[:, 512 + jc:512 + jc + PTILE],
                                     w2(t, 1), start=False, stop=not has_b2)
                    if has_b2:
                        c0 = PTILE + t * H3
                        nc.tensor.matmul(reg, B2R[0:1, 0:PTILE],
                                         B2R[0:1, c0:c0 + H3],
                                         start=False, stop=True)

            def stage_E3(i, eng):
                t, nt, _ = sched[i]
                N = nt * PTILE
                p3 = p3t[i]
                h3 = hpool.tile([128, 512], _BF16, tag="h3t", name="h3")
                h3t[i] = h3
                ev_relu(eng, h3[:, 0:N], p3[:, 0:N], None)
                p3t[i] = None
                h2t[i] = None

            def stage_D(i):
                t, nt, jg = sched[i]
                h3 = h3t[i]
                if i == first_st[t]:
                    G[0] = pgpool.tile([H3, B], _F32, tag="G", name="G")
                if "nogp" not in KDBG:
                    for j in range(nt):
                        jj = jg + j
                        nc.tensor.matmul(
                            G[0][:], h3[:, j * PTILE:(j + 1) * PTILE],
                            S2[:, jj * B:(jj + 1) * B],
                            start=(i == first_st[t] and j == 0),
                            stop=(i == last_st[t] and j == nt - 1),
                            skip_group_check=True)
                if i == last_st[t]:
                    nc.scalar.copy(GSB[:, t * B:(t + 1) * B], G[0][:])
                    G[0] = None
                h3t[i] = None

            # ---- software-pipelined main loop ----
            # evac engines alternate per emission iteration: the E1-engine
            # also takes E3 (small), the other takes E2; averages out.
            for i in range(n_st + 3):
                e = i % 2
                if i < n_st:
                    stage_A(i)
                    stage_E1(i, e)
                if 1 <= i <= n_st:
                    stage_B(i - 1)
                    stage_E2(i - 1, 1 - e)
                if 2 <= i <= n_st + 1:
                    stage_C(i - 2)
                    stage_E3(i - 2, e)
                if 3 <= i <= n_st + 2:
                    stage_D(i - 3)

            # ---- final: ship the per-type G matrices ----
            if "nogp" in KDBG:
                nc.vector.memset(GSB[:], 0.0)
            nc.sync.dma_start(out_d[:], GSB[:])
    nc.finalize()
    return nc


def kernel(**inputs):
    global LAST_RESULTS
    f1 = np.ascontiguousarray(np.asarray(inputs["frag1_layer"], np.float32))
    f2 = np.ascontiguousarray(np.asarray(inputs["frag2_layer"], np.float32))
    cx = np.ascontiguousarray(np.asarray(inputs["complex_layer"], np.float32))
    z1 = np.asarray(inputs["frag1_z"])
    z2 = np.asarray(inputs["frag2_z"])
    zc = np.asarray(inputs["complex_z"])
    W0 = np.asarray(inputs["W0"], np.float32)
    b0 = np.asarray(inputs["b0"], np.float32)
    W1 = np.asarray(inputs["W1"], np.float32)
    b1 = np.asarray(inputs["b1"], np.float32)
    W2 = np.asarray(inputs["W2"], np.float32)
    b2 = np.asarray(inputs["b2"], np.float32)
    Wout = np.asarray(inputs["Wout"], np.float32)
    bout = np.asarray(inputs["bout"], np.float32)

    np_dt = ml_dtypes.bfloat16
    has_b01 = bool(np.any(b0 != 0.0) or np.any(b1 != 0.0))
    has_b2 = bool(np.any(b2 != 0.0))

    x_all = np.concatenate([f1, f2, cx], axis=1)          # [B, Na, F]
    z_all = np.concatenate([z1, z2, zc], axis=1)          # [B, Na]
    Bn, Na, _ = x_all.shape
    assert Bn == B
    sgn = np.concatenate([
        np.full(f1.shape[1], -1.0, np.float32),
        np.full(f2.shape[1], -1.0, np.float32),
        np.full(cx.shape[1], 1.0, np.float32),
    ])

    xf = x_all.reshape(-1, F)
    zf = z_all.reshape(-1).astype(np.int64)
    bidx = np.repeat(np.arange(B), Na)
    sf = np.tile(sgn, B)

    order = np.argsort(zf, kind="stable")
    counts = np.bincount(zf, minlength=T)[:T]
    GRAN = NCORES * PTILE
    padded = -(-counts // GRAN) * GRAN
    k_t = (padded // GRAN).astype(int)
    n_core = int(padded.sum()) // NCORES
    ntt = n_core // PTILE

    # Per-core atom index lists; -1 marks padding.
    per_core = [[] for _ in range(NCORES)]
    pos = 0
    for t in range(T):
        ct, pt = int(counts[t]), int(padded[t])
        idx = order[pos:pos + ct]
        pos += ct
        if pt == 0:
            continue
        ip = np.full(pt, -1, np.int64)
        ip[:ct] = idx
        ip = ip.reshape(NCORES, pt // NCORES)
        for c in range(NCORES):
            per_core[c].append(ip[c])
    idx_cores = np.stack([np.concatenate(l) for l in per_core])  # [NC, n]

    valid = idx_cores >= 0
    safe = np.where(valid, idx_cores, 0)
    xg = xf[safe]
    xg[~valid] = 0.0
    xT = np.ascontiguousarray(xg.transpose(0, 2, 1)).astype(np_dt)  # [NC,F,n]

    # S[c, n, b] = sign * (batch == b)
    S = np.zeros((NCORES, n_core, B), np.float32)
    rows = sf[safe] * valid
    bcols = bidx[safe]
    S[np.arange(NCORES)[:, None], np.arange(n_core)[None, :], bcols] = rows

    # CONSTW: weights packed per type in the _TBLK layout
    CWh = np.zeros((PTILE, _WCOLS), np.float32)
    for t in range(T):
        base = t * _TBLK
        for k in range(2):
            for m in range(2):
                CWh[:, base + (k * 2 + m) * 128:base + (k * 2 + m + 1) * 128] = \
                    W0[t, 128 * k:128 * (k + 1), 128 * m:128 * (m + 1)]
                CWh[:, base + 512 + (k * 2 + m) * 128:
                    base + 512 + (k * 2 + m + 1) * 128] = \
                    W1[t, 128 * k:128 * (k + 1), 128 * m:128 * (m + 1)]
            CWh[:, base + 1024 + k * 128:base + 1024 + (k + 1) * 128] = \
                W2[t, 128 * k:128 * (k + 1), 0:128]
    CWh = np.ascontiguousarray(CWh).astype(np_dt)

    # CONSTF: 25 bias cols (b0 halves, b1 halves, b2 first half)
    CFh = np.zeros((PTILE, _FCOLS), np.float32)
    for t in range(T):
        CFh[:, t * 5 + 0] = b0[t, :128]
        CFh[:, t * 5 + 1] = b0[t, 128:]
        CFh[:, t * 5 + 2] = b1[t, :128]
        CFh[:, t * 5 + 3] = b1[t, 128:]
        CFh[:, t * 5 + 4] = b2[t, :128]

    bias_term = np.bincount(bidx, weights=(sf * bout[zf, 0]).astype(np.float64),
                            minlength=B)[:B]

    nc = _build(k_t, n_core, has_b01, has_b2)
    in_maps = []
    for c in range(NCORES):
        # S2[p, j*B + b] (tile-major)
        s2 = np.ascontiguousarray(
            S[c].reshape(ntt, PTILE, B).transpose(1, 0, 2)
        ).reshape(PTILE, ntt * B).astype(np_dt)
        m = {"xT": xT[c], "CONSTW": CWh, "S2": s2}
        if has_b01:
            m["CONSTF"] = CFh
        if has_b2:
            b2r = np.zeros((1, PTILE + T * H3), np.float32)
            b2r[0, :PTILE] = 1.0
            b2r[0, PTILE:] = b2[:, :H3].reshape(-1)
            m["B2R"] = b2r.astype(np_dt)
        in_maps.append(m)

    kw = {}
    if TRACE:
        kw = dict(trace=True, trace_cores=list(range(NCORES)))
    res = run_bass_kernel_spmd(nc, in_maps, core_ids=list(range(NCORES)), **kw)
    LAST_RESULTS = res

    # host fold: out[b] = sum_c sum_t sum_k Wout[t,k] * G[c,t,k,b] + bias
    out = bias_term.copy()
    wo = Wout[:, :, 0].astype(np.float64)  # [T, H3]
    for c in range(NCORES):
        g = res.results[c]["res"].reshape(PTILE, T, B).astype(np.float64)
        out += np.einsum("tk,ktb->b", wo, g[:H3])
    return out.astype(np.float32)[:, None]
